# revision 8
# baseline (speedup 1.0000x reference)
"""ALiBi multi-head causal attention on 8 TRN2 NeuronCores.

Sharding: core = b*4 + hg  (b in 0..1 batches, hg in 0..3 head-groups).
Each core computes 4 heads of one batch end-to-end (KQV projection for its
head-columns + causal ALiBi attention).  No collectives needed.

Per-core kernel layout:
  - xT   [D, S]   : x[b].T (bf16)  -- contraction dim D on partitions
  - wKQ  [D, 1024]: W columns for K(h0..h3) then Q(h0..h3) (bf16)
  - wV   [D, 512] : W columns for V(h0..h3) (bf16)
  - kqT  = (x W_kq)^T computed as [hd, S] per head (head-dim on partitions)
  - v    = x W_v computed as [S, hd] blocks
  - scores  = qT-block^T @ kT  -> [128, L] in PSUM, + m*(t-i) bias tile
  - probs   = exp(score + (-m*128*qi)) (per-q-block shift makes bias m*(t-s))
              with accum_out giving the row sum (no max-subtraction; exponent
              is bounded: alibi bias <= 0 in the causal region, raw scores O(10))
  - PV: PE-transpose prob chunks, matmul with v blocks, accumulate in PSUM,
        then scale rows by 1/rowsum.
"""

import sys

if "/opt/trn_rl_repo" not in sys.path:
    sys.path.insert(0, "/opt/trn_rl_repo")

import numpy as np
import ml_dtypes

import concourse.bass as bass
import concourse.mybir as mybir
from concourse import bacc
from concourse.tile import TileContext
from concourse.masks import make_identity
from concourse.bass_utils import run_bass_kernel_spmd

P = 128
S = 2048
D = 2048
HD = 128
NB = S // P            # 16 seq blocks
H_LOC = 4              # heads per core
NUM_HEADS = 16
SCALE = 1.0 / np.sqrt(HD)

F32 = mybir.dt.float32
BF16 = mybir.dt.bfloat16
I32 = mybir.dt.int32
AF = mybir.ActivationFunctionType
OP = mybir.AluOpType


def _alibi_slopes(num_heads=NUM_HEADS):
    base = (2.0 ** 8) ** (1.0 / num_heads)
    return np.asarray([1.0 / base ** (i + 1) for i in range(num_heads)], np.float32)


def build():
    nc = bacc.Bacc("TRN2", target_bir_lowering=False)

    xT_d = nc.declare_dram_parameter("xT", [D, S], BF16, isOutput=False)
    wKQ_d = nc.declare_dram_parameter("wKQ", [D, 8 * P], BF16, isOutput=False)
    wV_d = nc.declare_dram_parameter("wV", [D, H_LOC * HD], BF16, isOutput=False)
    bKQ_d = nc.declare_dram_parameter("bKQ", [P, 8], F32, isOutput=False)
    bV_d = nc.declare_dram_parameter("bV", [P, H_LOC * HD], F32, isOutput=False)
    biasH_d = nc.declare_dram_parameter("biasH", [H_LOC, P, S], F32, isOutput=False)
    negsh_d = nc.declare_dram_parameter("negsh", [P, H_LOC, NB], F32, isOutput=False)
    causal_d = nc.declare_dram_parameter("causal", [P, P], F32, isOutput=False)
    out_d = nc.declare_dram_parameter("out", [H_LOC, NB, P, HD], F32, isOutput=True)

    xT_t = xT_d.rearrange("(ko p) s -> p ko s", p=P)     # [128, 16, 2048]
    wKQ_t = wKQ_d.rearrange("(ko p) n -> p ko n", p=P)   # [128, 16, 1024]
    wV_t = wV_d.rearrange("(ko p) n -> p ko n", p=P)     # [128, 16, 512]

    with TileContext(nc) as tc:
        with (
            tc.tile_pool(name="const", bufs=1) as const,
            tc.tile_pool(name="resid", bufs=1) as resid,
            tc.tile_pool(name="stats", bufs=4) as stats,
            tc.tile_pool(name="psA", bufs=4, space="PSUM") as psA,
            tc.tile_pool(name="psT", bufs=2, space="PSUM") as psT,
            tc.tile_pool(name="psO", bufs=2, space="PSUM") as psO,
        ):
            # ---- constants ----
            ident = const.tile([P, P], BF16)
            make_identity(nc, ident)

            causal = const.tile([P, P], F32)  # additive: 0 if t<=s else -1e30
            nc.sync.dma_start(causal, causal_d[:])

            bkq_sb = const.tile([P, 8], F32)
            nc.sync.dma_start(bkq_sb, bKQ_d[:])
            bv_sb = const.tile([P, H_LOC * HD], F32)
            nc.sync.dma_start(bv_sb, bV_d[:])

            neg_shift = const.tile([P, H_LOC, NB], F32)  # -m_h * 128 * qi
            nc.sync.dma_start(neg_shift, negsh_d[:])

            # ---- residents ----
            kq_all = resid.tile([P, 8, S], BF16)       # [hd, (K h0..3 | Q h0..3), s]
            v_all = resid.tile([P, NB, H_LOC * HD], BF16)  # [si, so, j*128+d]

            # ---- phase 1: KQV projection ----
            with (
                tc.tile_pool(name="wpool", bufs=1) as wpool,
                tc.tile_pool(name="xpool", bufs=2) as xpool,
            ):
                wkq_sb = wpool.tile([P, 16, 8 * P], BF16)
                nc.sync.dma_start(wkq_sb, wKQ_t)
                wv_sb = wpool.tile([P, 16, H_LOC * HD], BF16)
                nc.sync.dma_start(wv_sb, wV_t)

                for nb in range(S // 512):
                    xc = xpool.tile([P, 16, 512], BF16, tag="xc")
                    nc.sync.dma_start(xc, xT_t[:, :, nb * 512 : (nb + 1) * 512])
                    for m in range(8):
                        ps = psA.tile([P, 512], F32, tag="ps")
                        for k in range(16):
                            nc.tensor.matmul(
                                ps,
                                lhsT=wkq_sb[:, k, m * P : (m + 1) * P],
                                rhs=xc[:, k, :],
                                start=(k == 0),
                                stop=(k == 15),
                            )
                        # kqT = psum * scale + bias  (scale folds 1/sqrt(hd) into q)
                        nc.scalar.activation(
                            kq_all[:, m, nb * 512 : (nb + 1) * 512],
                            ps,
                            AF.Identity,
                            bias=bkq_sb[:, m : m + 1],
                            scale=float(SCALE) if m >= 4 else 1.0,
                        )
                    for sub in range(4):
                        s_idx = nb * 4 + sub
                        psv = psA.tile([P, 512], F32, tag="ps")
                        for k in range(16):
                            nc.tensor.matmul(
                                psv,
                                lhsT=xc[:, k, sub * P : (sub + 1) * P],
                                rhs=wv_sb[:, k, :],
                                start=(k == 0),
                                stop=(k == 15),
                            )
                        nc.vector.tensor_tensor(
                            v_all[:, s_idx, :], psv, bv_sb, OP.add
                        )

            # ---- phase 2: attention ----
            with (
                tc.tile_pool(name="attn", bufs=2) as attn_pool,
                tc.tile_pool(name="biasp", bufs=2) as bias_pool,
            ):
                for j in range(H_LOC):
                    bias_h = bias_pool.tile([P, S], F32, tag="bias_h")
                    nc.sync.dma_start(bias_h, biasH_d[j])
                    for qi in range(NB):
                        L = (qi + 1) * P
                        nch = (L + 511) // 512
                        score = attn_pool.tile([P, S], F32, tag="score")
                        for c5 in range(nch):
                            w = min(512, L - c5 * 512)
                            ps = psA.tile([P, 512], F32, tag="ps")
                            nc.tensor.matmul(
                                ps[:, :w],
                                lhsT=kq_all[:, 4 + j, qi * P : (qi + 1) * P],
                                rhs=kq_all[:, j, c5 * 512 : c5 * 512 + w],
                                start=True,
                                stop=True,
                            )
                            nc.vector.tensor_tensor(
                                score[:, c5 * 512 : c5 * 512 + w],
                                ps[:, :w],
                                bias_h[:, c5 * 512 : c5 * 512 + w],
                                OP.add,
                            )
                        # causal mask on the diagonal block
                        nc.vector.tensor_tensor(
                            score[:, qi * P : L], score[:, qi * P : L], causal, OP.add
                        )
                        probs = attn_pool.tile([P, S], BF16, tag="probs")
                        rowsum = stats.tile([P, 1], F32, tag="rowsum")
                        nc.scalar.activation(
                            probs[:, :L],
                            score[:, :L],
                            AF.Exp,
                            bias=neg_shift[:, j, qi : qi + 1],
                            scale=1.0,
                            accum_out=rowsum,
                        )
                        # transpose prob chunks (PE) for the PV matmul
                        pt = attn_pool.tile([P, NB, P], BF16, tag="pt")
                        for c in range(qi + 1):
                            tps = psT.tile([P, P], BF16, tag="tps")
                            nc.tensor.transpose(
                                tps, probs[:, c * P : (c + 1) * P], ident
                            )
                            nc.vector.tensor_copy(pt[:, c, :], tps)
                        po = psO.tile([P, HD], F32, tag="po")
                        for c in range(qi + 1):
                            nc.tensor.matmul(
                                po,
                                lhsT=pt[:, c, :],
                                rhs=v_all[:, c, j * HD : (j + 1) * HD],
                                start=(c == 0),
                                stop=(c == qi),
                            )
                        recip = stats.tile([P, 1], F32, tag="recip")
                        nc.vector.reciprocal(recip, rowsum)
                        ob = attn_pool.tile([P, HD], F32, tag="ob")
                        nc.vector.tensor_scalar_mul(ob, po, recip)
                        nc.sync.dma_start(out_d[j, qi], ob)

    nc.finalize()
    return nc


_NC_CACHE = None


def _get_nc():
    global _NC_CACHE
    if _NC_CACHE is None:
        _NC_CACHE = build()
    return _NC_CACHE


def _make_in_maps(x, W_kqv, b_kqv):
    x = np.asarray(x, np.float32)
    W = np.asarray(W_kqv, np.float32)
    b = np.asarray(b_kqv, np.float32)
    slopes = _alibi_slopes()
    in_maps = []
    for core in range(8):
        bi, hg = divmod(core, 4)
        heads = [4 * hg + j for j in range(H_LOC)]
        xT = np.ascontiguousarray(x[bi].T).astype(ml_dtypes.bfloat16)
        wkq = np.concatenate(
            [W[:, h * HD : (h + 1) * HD] for h in heads]
            + [W[:, D + h * HD : D + (h + 1) * HD] for h in heads],
            axis=1,
        ).astype(ml_dtypes.bfloat16)
        wv = np.concatenate(
            [W[:, 2 * D + h * HD : 2 * D + (h + 1) * HD] for h in heads], axis=1
        ).astype(ml_dtypes.bfloat16)
        # bias columns: K h0..h3 then Q h0..h3; q-side prescaled by 1/sqrt(hd)
        bkq = np.stack(
            [b[h * HD : (h + 1) * HD] for h in heads]
            + [b[D + h * HD : D + (h + 1) * HD] * SCALE for h in heads],
            axis=1,
        ).astype(np.float32)
        bv = np.tile(
            np.concatenate(
                [b[2 * D + h * HD : 2 * D + (h + 1) * HD] for h in heads]
            )[None, :],
            (P, 1),
        ).astype(np.float32)
        # biasH[j, i, t] = m_h * (t - i);  negsh[p, j, qi] = -m_h * 128 * qi
        rel = (np.arange(S)[None, :] - np.arange(P)[:, None]).astype(np.float32)
        bias_h = (slopes[heads][:, None, None] * rel[None]).astype(np.float32)
        negsh = np.tile(
            (-slopes[heads][:, None] * (P * np.arange(NB))[None, :])[None],
            (P, 1, 1),
        ).astype(np.float32)
        causal = np.where(
            np.arange(P)[:, None] >= np.arange(P)[None, :], 0.0, -1e30
        ).astype(np.float32)
        in_maps.append(
            dict(
                xT=xT, wKQ=wkq, wV=wv, bKQ=bkq, bV=bv,
                biasH=bias_h, negsh=negsh, causal=causal,
            )
        )
    return in_maps


def run(inputs, trace=False, **kw):
    nc = _get_nc()
    in_maps = _make_in_maps(inputs["x"], inputs["W_kqv"], inputs["b_kqv"])
    bkr = run_bass_kernel_spmd(nc, in_maps, core_ids=list(range(8)), trace=trace, **kw)
    B = 2
    out = np.empty((B, NUM_HEADS, S, HD), np.float32)
    for core in range(8):
        bi, hg = divmod(core, 4)
        o = np.asarray(bkr.results[core]["out"])  # [4, 16, 128, 128]
        for j in range(H_LOC):
            out[bi, 4 * hg + j] = o[j].reshape(S, HD)
    return out, bkr


def kernel(x, W_kqv, b_kqv):
    out, _ = run({"x": x, "W_kqv": W_kqv, "b_kqv": b_kqv})
    return out
